# revision 13
# baseline (speedup 1.0000x reference)
"""BitLinear (bit-decoded weights + STE quant) Trainium2 kernel, v3.

y = x @ W^T + b, where
  W = decode_bits(bweight, wsign) * scale,  b = decode_bits(bbias, bsign)
      * biasscale; STE forward == identity on the already-quantized decode.

Decode: n = sum_k bits[..., k] * 2^(7-k) (exact integers 0..255),
        W = n * sign(wsign) * (scale/255).

Device strategy per core (2 token-groups x 4 out-feature-groups grid):
  - bit-plane summation runs INSIDE the DMA engines: host stores plane k
    as an fp8 power-of-two relabeling of the bit; per out-block the
    device issues two 4-deep chains of SWDGE accumulate-DMAs
    (accum_op=add, fp8 dest): hi = 8*b0+4*b1+2*b2+b3, lo =
    8*b4+4*b5+2*b6+b7 -- all partial sums <= 15, exact in fp8e4m3 -- so
    the bit summation runs in the SDMA CCE units with near-zero
    Vector-engine cost. Each link is [128, 2048] = 256 KiB; the CCE
    path was measured to crash above 2048 elements/partition per DMA.
  - DVE forms n = 16*hi + lo (exact fp16 integers <= 255); ACT computes
    sign(wsign) in place; one DVE multiply forms W_int = n * sign.
  - matmul: psum[o=128, t=512] += W_int^T[i,o-blk] @ x^T[i,t-chunk] in
    fp16, fp32 PSUM accumulation; t-group-major with triple-buffered x
    streaming, 8 PSUM banks.
  - eviction on ACT: y^T = Identity(psum * (scale/255) + bias_o), bf16
    output (host upcasts to f32; 2^-9 rounding ~10x under tolerance).
  - weights (and bias/scalars) are double-buffered across repeat
    iterations so the decode of iteration k+1 overlaps the matmuls of
    iteration k: the in-NEFF repeat loop then measures the kernel's
    steady-state (pipelined) throughput.

Distribution over 8 NeuronCores: no collectives - each core writes its
own y^T shard, host reassembles.

Host-side work is layout/precision only: transposes, shard slicing, and
dtype/scale-exponent conversion. All of the module's arithmetic (bit
summation, sign, scaling, matmul, bias) runs on the device.
"""

import numpy as np

import concourse.mybir as mybir
import concourse.tile as tile
from concourse import bacc
from concourse import bass_utils

# ---- problem constants (hardcoded per contract) ----
B, S, IN, OUT, NB = 4, 2048, 2048, 2048, 8
T = B * S                      # 8192 tokens
P = 128                        # partitions
P_T, P_O = 2, 4                # token-parallel x out-feature-parallel grid
N_CORES = P_T * P_O
T_SH = T // P_T                # 4096
O_SH = OUT // P_O              # 512
KB = IN // P                   # 16 contraction blocks
OB = O_SH // P                 # 4 out blocks
TGW = 512                      # t-group width
NT = T_SH // TGW               # 8 t-groups per core
WCOLS = KB * P                 # 2048 weight cols per out-block (CCE limit)

F32 = mybir.dt.float32
FP16 = mybir.dt.float16
BF16 = mybir.dt.bfloat16
FP8 = mybir.dt.float8e4
AL = mybir.AluOpType
IDENT = mybir.ActivationFunctionType.Identity

_CACHE = {}

# timing-bisect switches (do not affect correctness-critical default path)
import os as _os
_NO_ACCUM = _os.environ.get("K_NO_ACCUM", "0") == "1"   # plain links, garbage W
_Y_SYNC = _os.environ.get("K_Y_SYNC", "0") == "1"       # y DMA on sync queue
_NO_DECODE = _os.environ.get("K_NO_DECODE", "0") == "1"  # skip chains+sign
_PLAIN_LINKS = _os.environ.get("K_PLAIN_LINKS", "0") == "1"  # unchained links


def _pairs(ap):
    """Split the last (fast) axis of a [..., 2n] AP into even/odd views."""
    v = ap.rearrange("p (c two) -> p c two", two=2)
    return v[:, :, 0], v[:, :, 1]


def _build_nc(repeats=1):
    nc = bacc.Bacc("TRN2", target_bir_lowering=False, debug=False,
                   num_devices=N_CORES)

    xgd = nc.dram_tensor("xg", [NT * P, KB * TGW], FP16,
                         kind="ExternalInput").ap()
    bits = nc.dram_tensor("bits", [P, NB * OB * WCOLS], FP8,
                          kind="ExternalInput").ap()
    wsd = nc.dram_tensor("ws", [P, OB * WCOLS], BF16,
                         kind="ExternalInput").ap()
    bb = nc.dram_tensor("bb", [P, OB * NB], F32, kind="ExternalInput").ap()
    bs = nc.dram_tensor("bs", [P, OB], F32, kind="ExternalInput").ap()
    scl = nc.dram_tensor("scl", [P, 1], F32, kind="ExternalInput").ap()
    bscl = nc.dram_tensor("bscl", [P, 1], F32, kind="ExternalInput").ap()
    y = nc.dram_tensor("y", [O_SH, T_SH], BF16, kind="ExternalOutput").ap()

    with tile.TileContext(nc) as tc:
      with tc.tile_pool(name="w", bufs=1) as wpool, \
           tc.tile_pool(name="xs", bufs=1) as xpool, \
           tc.tile_pool(name="dec", bufs=1) as dec, \
           tc.tile_pool(name="yb", bufs=1) as ypool, \
           tc.tile_pool(name="psum", bufs=1, space="PSUM") as psum_pool:

        for rep in range(repeats):
            par = rep % 2   # parity suffix: double-buffer rep-crossing state

            # ---- scalars ----
            scl_sb = dec.tile([P, 1], F32, name=f"scl_sb{par}")
            nc.sync.dma_start(out=scl_sb, in_=scl)
            bscl_sb = dec.tile([P, 1], F32, name=f"bscl_sb{par}")
            nc.sync.dma_start(out=bscl_sb, in_=bscl)
            s255 = dec.tile([P, 1], F32, name=f"s255_{par}")
            nc.vector.tensor_scalar_mul(s255, scl_sb, 1.0 / 255.0)
            bs255 = dec.tile([P, 1], F32, name=f"bs255_{par}")
            nc.vector.tensor_scalar_mul(bs255, bscl_sb, 1.0 / 255.0)

            # ---- bias decode: bias_col [128, OB] (o on partitions) ----
            bb_sb = dec.tile([P, OB * NB], F32, name=f"bb_sb{par}")
            nc.sync.dma_start(out=bb_sb, in_=bb)
            bs_sb = dec.tile([P, OB], F32, name=f"bs_sb{par}")
            nc.sync.dma_start(out=bs_sb, in_=bs)
            e, o = _pairs(bb_sb)
            bl1 = dec.tile([P, OB * 4], F32, name=f"bl1_{par}")
            nc.vector.scalar_tensor_tensor(out=bl1, in0=e, scalar=2.0,
                                           in1=o, op0=AL.mult, op1=AL.add)
            e, o = _pairs(bl1)
            bl2 = dec.tile([P, OB * 2], F32, name=f"bl2_{par}")
            nc.vector.scalar_tensor_tensor(out=bl2, in0=e, scalar=4.0,
                                           in1=o, op0=AL.mult, op1=AL.add)
            e, o = _pairs(bl2)
            bl3 = dec.tile([P, OB], F32, name=f"bl3_{par}")
            nc.vector.scalar_tensor_tensor(out=bl3, in0=e, scalar=16.0,
                                           in1=o, op0=AL.mult, op1=AL.add)
            bsg = dec.tile([P, OB], F32, name=f"bsg{par}")
            nc.scalar.sign(bsg, bs_sb)
            bias_col = dec.tile([P, OB], F32, name=f"bias_col{par}")
            nc.vector.scalar_tensor_tensor(out=bias_col, in0=bl3,
                                           scalar=bs255, in1=bsg,
                                           op0=AL.mult, op1=AL.mult)

            # ---- weight decode via accumulate-DMA chains, per out-block ----
            Ws = []
            for ob in range(OB):
                if _NO_DECODE:
                    Wg = wpool.tile([P, WCOLS], FP16, name=f"W{ob}_{par}")
                    nc.vector.memset(Wg, 1.0)
                    Ws.append(Wg.rearrange("p (kb o) -> p kb o", kb=KB))
                    continue
                if _PLAIN_LINKS:
                    base = ob * NB * WCOLS
                    for j in range(NB):
                        gt = dec.tile([P, WCOLS], FP8, tag="gl", bufs=4)
                        off = base + j * WCOLS
                        nc.gpsimd.dma_start(out=gt,
                                            in_=bits[:, off:off + WCOLS])
                    Wg = wpool.tile([P, WCOLS], FP16, name=f"W{ob}_{par}")
                    nc.vector.memset(Wg, 1.0)
                    Ws.append(Wg.rearrange("p (kb o) -> p kb o", kb=KB))
                    continue
                hi = dec.tile([P, WCOLS], FP8, name=f"hi{ob}")
                lo = dec.tile([P, WCOLS], FP8, name=f"lo{ob}")
                base = ob * NB * WCOLS
                for j in range(4):
                    op = AL.bypass if _NO_ACCUM else (
                        AL.add if j else AL.bypass)
                    off = base + 2 * j * WCOLS
                    nc.gpsimd.dma_start(
                        out=hi, in_=bits[:, off:off + WCOLS], accum_op=op)
                    off += WCOLS
                    nc.gpsimd.dma_start(
                        out=lo, in_=bits[:, off:off + WCOLS], accum_op=op)
                wsg = dec.tile([P, WCOLS], BF16, name=f"wsg{ob}")
                nc.scalar.dma_start(
                    out=wsg, in_=wsd[:, ob * WCOLS:(ob + 1) * WCOLS])
                nc.scalar.sign(wsg, wsg)          # in place: +-1
                Wg = wpool.tile([P, WCOLS], FP16, name=f"W{ob}_{par}")
                nc.vector.scalar_tensor_tensor(out=Wg, in0=hi, scalar=16.0,
                                               in1=lo, op0=AL.mult,
                                               op1=AL.add)
                nc.vector.tensor_tensor(out=Wg, in0=Wg, in1=wsg,
                                        op=AL.mult)
                Ws.append(Wg.rearrange("p (kb o) -> p kb o", kb=KB))

            # ---- main matmul: t-group-major, x triple-buffered ----
            def load_xg(g):
                xt = xpool.tile([P, KB * TGW], FP16, tag="xg",
                                name=f"xg{g}_{rep}", bufs=3)
                nc.sync.dma_start(out=xt, in_=xgd[g * P:(g + 1) * P, :])
                return xt

            xtiles = {g: load_xg(g) for g in range(min(3, NT))}
            for g in range(NT):
                xg3 = xtiles[g].rearrange("p (kb t) -> p kb t", kb=KB)
                ybuf = ypool.tile([P, OB * TGW], BF16, tag="yb", bufs=2)
                yb3 = ybuf.rearrange("p (ob t) -> p ob t", ob=OB)
                for ob in range(OB):
                    ps = psum_pool.tile([P, TGW], F32, tag="mm", bufs=8)
                    for kb in range(KB):
                        nc.tensor.matmul(
                            ps,
                            Ws[ob][:, kb],
                            xg3[:, kb],
                            start=(kb == 0),
                            stop=(kb == KB - 1),
                        )
                    # y^T tile = psum * (scale/255) + bias_o   (ACT)
                    nc.scalar.activation(
                        out=yb3[:, ob], in_=ps, func=IDENT,
                        bias=bias_col[:, ob:ob + 1], scale=s255)
                y_eng = nc.sync if _Y_SYNC else nc.scalar
                y_eng.dma_start(
                    out=y.rearrange("(ob p) t -> p ob t", p=P)[
                        :, :, g * TGW:(g + 1) * TGW],
                    in_=yb3,
                )
                if g + 3 < NT:
                    xtiles[g + 3] = load_xg(g + 3)

    nc.compile()
    return nc


def _shard_inputs(x, bweight, wsign, scale, bbias, bsign, biasscale):
    fp8_np = mybir.dt.np(FP8)
    bf16_np = mybir.dt.np(BF16)

    x2 = np.asarray(x, dtype=np.float32).reshape(T, IN)
    bwf = np.asarray(bweight, dtype=np.float32)
    wsf = np.asarray(wsign, dtype=np.float32)
    bbias = np.asarray(bbias, dtype=np.float32)
    bsign = np.asarray(bsign, dtype=np.float32)

    scl_rep = np.full((P, 1), np.asarray(scale).reshape(-1)[0],
                      dtype=np.float32)
    bscl_rep = np.full((P, 1), np.asarray(biasscale).reshape(-1)[0],
                       dtype=np.float32)

    def col_major(a):
        # [IN, P] -> partition-major [P, KB*P]
        return np.ascontiguousarray(
            a.reshape(KB, P, P).transpose(1, 0, 2).reshape(P, KB * P))

    # hi/lo chains: plane j and 4+j both scaled by 8>>j so every partial
    # sum is <= 15 (exact in fp8e4m3)
    sc = (2.0 ** np.arange(3, -1, -1)).astype(np.float32)  # 8,4,2,1

    o_maps = []
    for o_grp in range(P_O):
        osl = slice(o_grp * O_SH, (o_grp + 1) * O_SH)
        bw_sh = bwf[osl]                              # [O_SH, IN, NB]
        wT = wsf[osl].T                               # [IN, O_SH]
        bits_chunks = []
        ws_chunks = []
        for ob in range(OB):
            csl = slice(ob * P, (ob + 1) * P)
            for j in range(4):
                for half in (0, 4):
                    bits_chunks.append(col_major(np.ascontiguousarray(
                        (bw_sh[csl, :, half + j] * sc[j])
                        .astype(fp8_np).T)))
            ws_chunks.append(col_major(np.ascontiguousarray(wT[:, csl])))
        o_maps.append({
            "bits": np.concatenate(bits_chunks, axis=1),
            "ws": np.concatenate(ws_chunks, axis=1).astype(bf16_np),
            "bb": np.ascontiguousarray(
                bbias[osl].reshape(OB, P, NB).transpose(1, 0, 2)
                .reshape(P, OB * NB)),
            "bs": np.ascontiguousarray(bsign[osl].reshape(OB, P).T),
            "scl": scl_rep,
            "bscl": bscl_rep,
        })

    in_maps = [None] * N_CORES
    for t_grp in range(P_T):
        tsl = slice(t_grp * T_SH, (t_grp + 1) * T_SH)
        xs = x2[tsl]                                  # [T_SH, IN]
        xg_np = np.ascontiguousarray(
            xs.reshape(NT, TGW, KB, P).transpose(0, 3, 2, 1)
            .reshape(NT * P, KB * TGW).astype(np.float16))
        for o_grp in range(P_O):
            c = t_grp * P_O + o_grp
            in_maps[c] = dict(o_maps[o_grp], xg=xg_np)
    return in_maps


def kernel(x, bweight, wsign, scale, bbias, bsign, biasscale):
    if "nc" not in _CACHE:
        _CACHE["nc"] = _build_nc()
    nc = _CACHE["nc"]
    in_maps = _shard_inputs(x, bweight, wsign, scale, bbias, bsign, biasscale)
    res = bass_utils.run_bass_kernel_spmd(
        nc, in_maps, core_ids=list(range(N_CORES)))
    Y = np.empty((T, OUT), dtype=np.float32)
    for c in range(N_CORES):
        t_grp, o_grp = c // P_O, c % P_O
        Y[t_grp * T_SH:(t_grp + 1) * T_SH,
          o_grp * O_SH:(o_grp + 1) * O_SH] = \
            res.results[c]["y"].T.astype(np.float32)
    return Y.reshape(B, S, OUT)


# revision 17
# speedup vs baseline: 1.4754x; 1.4754x over previous
"""BitLinear (bit-decoded weights + STE quant) Trainium2 kernel, v3.

y = x @ W^T + b, where
  W = decode_bits(bweight, wsign) * scale,  b = decode_bits(bbias, bsign)
      * biasscale; STE forward == identity on the already-quantized decode.

Decode: n = sum_k bits[..., k] * 2^(7-k) (exact integers 0..255),
        W = n * sign(wsign) * (scale/255).

Device strategy per core (2 token-groups x 4 out-feature-groups grid):
  - bit-plane summation runs INSIDE the DMA engines: host stores plane k
    as an fp8 power-of-two relabeling of the bit; per out-block the
    device issues two 4-deep chains of SWDGE accumulate-DMAs
    (accum_op=add, fp8 dest): hi = 8*b0+4*b1+2*b2+b3, lo =
    8*b4+4*b5+2*b6+b7 -- all partial sums <= 15, exact in fp8e4m3 -- so
    the bit summation runs in the SDMA CCE units with near-zero
    Vector-engine cost. Each link is [128, 2048] = 256 KiB; the CCE
    path was measured to crash above 2048 elements/partition per DMA.
  - DVE forms n = 16*hi + lo (exact fp16 integers <= 255); ACT computes
    sign(wsign) in place; one DVE multiply forms W_int = n * sign.
  - matmul: psum[o=128, t=512] += W_int^T[i,o-blk] @ x^T[i,t-chunk] in
    fp16, fp32 PSUM accumulation; t-group-major with triple-buffered x
    streaming, 8 PSUM banks.
  - eviction on ACT: y^T = Identity(psum * (scale/255) + bias_o), bf16
    output (host upcasts to f32; 2^-9 rounding ~10x under tolerance).
  - weights (and bias/scalars) are double-buffered across repeat
    iterations so the decode of iteration k+1 overlaps the matmuls of
    iteration k: the in-NEFF repeat loop then measures the kernel's
    steady-state (pipelined) throughput.

Distribution over 8 NeuronCores: no collectives - each core writes its
own y^T shard, host reassembles.

Host-side work is layout/precision only: transposes, shard slicing, and
dtype/scale-exponent conversion. All of the module's arithmetic (bit
summation, sign, scaling, matmul, bias) runs on the device.
"""

import numpy as np

import concourse.mybir as mybir
import concourse.tile as tile
from concourse import bacc
from concourse import bass_utils

# ---- problem constants (hardcoded per contract) ----
B, S, IN, OUT, NB = 4, 2048, 2048, 2048, 8
T = B * S                      # 8192 tokens
P = 128                        # partitions
P_T, P_O = 2, 4                # token-parallel x out-feature-parallel grid
N_CORES = P_T * P_O
T_SH = T // P_T                # 4096
O_SH = OUT // P_O              # 512
KB = IN // P                   # 16 contraction blocks
OB = O_SH // P                 # 4 out blocks
TGW = 512                      # t-group width
NT = T_SH // TGW               # 8 t-groups per core
WCOLS = KB * P                 # 2048 weight cols per out-block (CCE limit)

F32 = mybir.dt.float32
FP16 = mybir.dt.float16
BF16 = mybir.dt.bfloat16
FP8 = mybir.dt.float8e4
AL = mybir.AluOpType
IDENT = mybir.ActivationFunctionType.Identity

_CACHE = {}

# timing-bisect switches (do not affect correctness-critical default path)
import os as _os
_NO_ACCUM = _os.environ.get("K_NO_ACCUM", "0") == "1"   # plain links, garbage W
_Y_SYNC = _os.environ.get("K_Y_SYNC", "0") == "1"       # y DMA on sync queue
_NO_DECODE = _os.environ.get("K_NO_DECODE", "0") == "1"  # skip chains+sign
_PLAIN_LINKS = _os.environ.get("K_PLAIN_LINKS", "0") == "1"  # unchained links
_NO_X = _os.environ.get("K_NO_X", "0") == "1"     # static x (no streaming)
_NO_Y = _os.environ.get("K_NO_Y", "0") == "1"     # skip y writeback


def _pairs(ap):
    """Split the last (fast) axis of a [..., 2n] AP into even/odd views."""
    v = ap.rearrange("p (c two) -> p c two", two=2)
    return v[:, :, 0], v[:, :, 1]


def _build_nc(repeats=1):
    nc = bacc.Bacc("TRN2", target_bir_lowering=False, debug=False,
                   num_devices=N_CORES)

    xgd = nc.dram_tensor("xg", [NT * P, KB * TGW], FP16,
                         kind="ExternalInput").ap()
    bits = nc.dram_tensor("bits", [P, NB * OB * WCOLS], FP8,
                          kind="ExternalInput").ap()
    wsd = nc.dram_tensor("ws", [P, OB * WCOLS], BF16,
                         kind="ExternalInput").ap()
    bb = nc.dram_tensor("bb", [P, OB * NB], F32, kind="ExternalInput").ap()
    bs = nc.dram_tensor("bs", [P, OB], F32, kind="ExternalInput").ap()
    scl = nc.dram_tensor("scl", [P, 1], F32, kind="ExternalInput").ap()
    bscl = nc.dram_tensor("bscl", [P, 1], F32, kind="ExternalInput").ap()
    y = nc.dram_tensor("y", [O_SH, T_SH], BF16, kind="ExternalOutput").ap()

    with tile.TileContext(nc) as tc:
      with tc.tile_pool(name="w", bufs=1) as wpool, \
           tc.tile_pool(name="xs", bufs=1) as xpool, \
           tc.tile_pool(name="dec", bufs=1) as dec, \
           tc.tile_pool(name="yb", bufs=1) as ypool, \
           tc.tile_pool(name="psum", bufs=1, space="PSUM") as psum_pool:

        xstatic = None
        if _NO_X and repeats:
            xstatic = xpool.tile([P, KB * TGW], FP16, name="xstatic")
            nc.sync.dma_start(out=xstatic, in_=xgd[0:P, :])

        for rep in range(repeats):
            par = rep % 2   # parity suffix: double-buffer rep-crossing state

            # ---- scalars ----
            scl_sb = dec.tile([P, 1], F32, name=f"scl_sb{par}")
            nc.sync.dma_start(out=scl_sb, in_=scl)
            bscl_sb = dec.tile([P, 1], F32, name=f"bscl_sb{par}")
            nc.sync.dma_start(out=bscl_sb, in_=bscl)
            s255 = dec.tile([P, 1], F32, name=f"s255_{par}")
            nc.vector.tensor_scalar_mul(s255, scl_sb, 1.0 / 255.0)
            bs255 = dec.tile([P, 1], F32, name=f"bs255_{par}")
            nc.vector.tensor_scalar_mul(bs255, bscl_sb, 1.0 / 255.0)

            # ---- bias decode: bias_col [128, OB] (o on partitions) ----
            bb_sb = dec.tile([P, OB * NB], F32, name=f"bb_sb{par}")
            nc.sync.dma_start(out=bb_sb, in_=bb)
            bs_sb = dec.tile([P, OB], F32, name=f"bs_sb{par}")
            nc.sync.dma_start(out=bs_sb, in_=bs)
            e, o = _pairs(bb_sb)
            bl1 = dec.tile([P, OB * 4], F32, name=f"bl1_{par}")
            nc.vector.scalar_tensor_tensor(out=bl1, in0=e, scalar=2.0,
                                           in1=o, op0=AL.mult, op1=AL.add)
            e, o = _pairs(bl1)
            bl2 = dec.tile([P, OB * 2], F32, name=f"bl2_{par}")
            nc.vector.scalar_tensor_tensor(out=bl2, in0=e, scalar=4.0,
                                           in1=o, op0=AL.mult, op1=AL.add)
            e, o = _pairs(bl2)
            bl3 = dec.tile([P, OB], F32, name=f"bl3_{par}")
            nc.vector.scalar_tensor_tensor(out=bl3, in0=e, scalar=16.0,
                                           in1=o, op0=AL.mult, op1=AL.add)
            bsg = dec.tile([P, OB], F32, name=f"bsg{par}")
            nc.scalar.sign(bsg, bs_sb)
            bias_col = dec.tile([P, OB], F32, name=f"bias_col{par}")
            nc.vector.scalar_tensor_tensor(out=bias_col, in0=bl3,
                                           scalar=bs255, in1=bsg,
                                           op0=AL.mult, op1=AL.mult)

            # ---- weight decode via accumulate-DMA chains, per out-block ----
            Ws = []
            for ob in range(OB):
                if _NO_DECODE:
                    Wg = wpool.tile([P, WCOLS], FP16, name=f"W{ob}_{par}")
                    nc.vector.memset(Wg, 1.0)
                    Ws.append(Wg.rearrange("p (kb o) -> p kb o", kb=KB))
                    continue
                if _PLAIN_LINKS:
                    base = ob * NB * WCOLS
                    for j in range(NB):
                        gt = dec.tile([P, WCOLS], FP8, tag="gl", bufs=4)
                        off = base + j * WCOLS
                        nc.gpsimd.dma_start(out=gt,
                                            in_=bits[:, off:off + WCOLS])
                    Wg = wpool.tile([P, WCOLS], FP16, name=f"W{ob}_{par}")
                    nc.vector.memset(Wg, 1.0)
                    Ws.append(Wg.rearrange("p (kb o) -> p kb o", kb=KB))
                    continue
                hi = dec.tile([P, WCOLS], FP8, name=f"hi{ob}")
                lo = dec.tile([P, WCOLS], FP8, name=f"lo{ob}")
                base = ob * NB * WCOLS
                for j in range(4):
                    op = AL.bypass if _NO_ACCUM else (
                        AL.add if j else AL.bypass)
                    off = base + 2 * j * WCOLS
                    nc.gpsimd.dma_start(
                        out=hi, in_=bits[:, off:off + WCOLS], accum_op=op)
                    off += WCOLS
                    nc.gpsimd.dma_start(
                        out=lo, in_=bits[:, off:off + WCOLS], accum_op=op)
                wsg = dec.tile([P, WCOLS], BF16, name=f"wsg{ob}")
                nc.scalar.dma_start(
                    out=wsg, in_=wsd[:, ob * WCOLS:(ob + 1) * WCOLS])
                nc.scalar.sign(wsg, wsg)          # in place: +-1
                Wg = wpool.tile([P, WCOLS], FP16, name=f"W{ob}_{par}")
                nc.vector.scalar_tensor_tensor(out=Wg, in0=hi, scalar=16.0,
                                               in1=lo, op0=AL.mult,
                                               op1=AL.add)
                nc.vector.tensor_tensor(out=Wg, in0=Wg, in1=wsg,
                                        op=AL.mult)
                Ws.append(Wg.rearrange("p (kb o) -> p kb o", kb=KB))

            # ---- main matmul: t-group-major, x triple-buffered ----
            def load_xg(g):
                xt = xpool.tile([P, KB * TGW], FP16, tag="xg",
                                name=f"xg{g}_{rep}", bufs=3)
                nc.sync.dma_start(out=xt, in_=xgd[g * P:(g + 1) * P, :])
                return xt

            if _NO_X:
                xtiles = {g: xstatic for g in range(NT)}
            else:
                xtiles = {g: load_xg(g) for g in range(min(3, NT))}
            for g in range(NT):
                xg3 = xtiles[g].rearrange("p (kb t) -> p kb t", kb=KB)
                ybuf = ypool.tile([P, OB * TGW], BF16, tag="yb", bufs=2)
                yb3 = ybuf.rearrange("p (ob t) -> p ob t", ob=OB)
                for ob in range(OB):
                    ps = psum_pool.tile([P, TGW], F32, tag="mm", bufs=8)
                    for kb in range(KB):
                        nc.tensor.matmul(
                            ps,
                            Ws[ob][:, kb],
                            xg3[:, kb],
                            start=(kb == 0),
                            stop=(kb == KB - 1),
                        )
                    # y^T tile = psum * (scale/255) + bias_o   (ACT)
                    nc.scalar.activation(
                        out=yb3[:, ob], in_=ps, func=IDENT,
                        bias=bias_col[:, ob:ob + 1], scale=s255)
                if not _NO_Y:
                    y_eng = nc.sync if _Y_SYNC else nc.scalar
                    y_eng.dma_start(
                        out=y.rearrange("(ob p) t -> p ob t", p=P)[
                            :, :, g * TGW:(g + 1) * TGW],
                        in_=yb3,
                    )
                if not _NO_X and g + 3 < NT:
                    xtiles[g + 3] = load_xg(g + 3)

    nc.compile()
    return nc


def _shard_inputs(x, bweight, wsign, scale, bbias, bsign, biasscale):
    fp8_np = mybir.dt.np(FP8)
    bf16_np = mybir.dt.np(BF16)

    x2 = np.asarray(x, dtype=np.float32).reshape(T, IN)
    bwf = np.asarray(bweight, dtype=np.float32)
    wsf = np.asarray(wsign, dtype=np.float32)
    bbias = np.asarray(bbias, dtype=np.float32)
    bsign = np.asarray(bsign, dtype=np.float32)

    scl_rep = np.full((P, 1), np.asarray(scale).reshape(-1)[0],
                      dtype=np.float32)
    bscl_rep = np.full((P, 1), np.asarray(biasscale).reshape(-1)[0],
                       dtype=np.float32)

    def col_major(a):
        # [IN, P] -> partition-major [P, KB*P]
        return np.ascontiguousarray(
            a.reshape(KB, P, P).transpose(1, 0, 2).reshape(P, KB * P))

    # hi/lo chains: plane j and 4+j both scaled by 8>>j so every partial
    # sum is <= 15 (exact in fp8e4m3)
    sc = (2.0 ** np.arange(3, -1, -1)).astype(np.float32)  # 8,4,2,1

    o_maps = []
    for o_grp in range(P_O):
        osl = slice(o_grp * O_SH, (o_grp + 1) * O_SH)
        bw_sh = bwf[osl]                              # [O_SH, IN, NB]
        wT = wsf[osl].T                               # [IN, O_SH]
        bits_chunks = []
        ws_chunks = []
        for ob in range(OB):
            csl = slice(ob * P, (ob + 1) * P)
            for j in range(4):
                for half in (0, 4):
                    bits_chunks.append(col_major(np.ascontiguousarray(
                        (bw_sh[csl, :, half + j] * sc[j])
                        .astype(fp8_np).T)))
            ws_chunks.append(col_major(np.ascontiguousarray(wT[:, csl])))
        o_maps.append({
            "bits": np.concatenate(bits_chunks, axis=1),
            "ws": np.concatenate(ws_chunks, axis=1).astype(bf16_np),
            "bb": np.ascontiguousarray(
                bbias[osl].reshape(OB, P, NB).transpose(1, 0, 2)
                .reshape(P, OB * NB)),
            "bs": np.ascontiguousarray(bsign[osl].reshape(OB, P).T),
            "scl": scl_rep,
            "bscl": bscl_rep,
        })

    in_maps = [None] * N_CORES
    for t_grp in range(P_T):
        tsl = slice(t_grp * T_SH, (t_grp + 1) * T_SH)
        xs = x2[tsl]                                  # [T_SH, IN]
        xg_np = np.ascontiguousarray(
            xs.reshape(NT, TGW, KB, P).transpose(0, 3, 2, 1)
            .reshape(NT * P, KB * TGW).astype(np.float16))
        for o_grp in range(P_O):
            c = t_grp * P_O + o_grp
            in_maps[c] = dict(o_maps[o_grp], xg=xg_np)
    return in_maps


def kernel(x, bweight, wsign, scale, bbias, bsign, biasscale):
    if "nc" not in _CACHE:
        _CACHE["nc"] = _build_nc()
    nc = _CACHE["nc"]
    in_maps = _shard_inputs(x, bweight, wsign, scale, bbias, bsign, biasscale)
    res = bass_utils.run_bass_kernel_spmd(
        nc, in_maps, core_ids=list(range(N_CORES)))
    Y = np.empty((T, OUT), dtype=np.float32)
    for c in range(N_CORES):
        t_grp, o_grp = c // P_O, c % P_O
        Y[t_grp * T_SH:(t_grp + 1) * T_SH,
          o_grp * O_SH:(o_grp + 1) * O_SH] = \
            res.results[c]["y"].T.astype(np.float32)
    return Y.reshape(B, S, OUT)
